# revision 23
# baseline (speedup 1.0000x reference)
"""MoE (noisy top-2 routing) Trainium2 kernel.

Strategy (expert parallelism, per sharding hint):
  - Host: compute gating logits + top-2 + softmax gates (cheap: T x E),
    gather each expert's tokens into a padded [capacity, D] batch.
  - Device (8 cores, 2 experts/core): per expert FFN
        hT = relu(W1^T @ x_e^T + b1)        (fp16 in, fp32 PSUM)
        yT = W2 @ hT  (transposed output)   (fp16 in, fp32 PSUM)
    Both matmuls keep the TOKEN axis as the streamed free dimension, so
    padded-capacity remainders never occupy the PE partition dim (no
    half-empty 512-column passes). b1 is applied for free in the ScalarE
    relu pass (per-partition bias). Outputs are stored fp16, transposed
    [D, C]; the gate scale and b2 are applied on host during combine.
  - Startup: inputs are split into priority-ordered DMA chunks on the
    sync HWDGE ring (first slot's x kd-chunks interleaved with W1
    mh-chunks in consumption order, then W2, then the second slot) so
    the first matmul fires after ~0.5MB instead of the full 19MB and the
    PE chases the arriving stream. Output DMAs ride the separate scalar
    HWDGE ring (one merged store per dp group; the final store goes out
    on the idle sync ring with its cast split across ScalarE/VectorE).
    Dummy matmuls on a zeroed tile pre-warm the PE HAM clock gate during
    the DMA wait, sized to finish just as the first real data lands.
    NOTE: keep every matmul operand slice <=3D — 4D tile slices degrade
    PE streaming from N/2.4GHz to N/2.0GHz per column (measured).

Expert pairing: experts are sorted by token count and paired
largest-with-smallest; slot 0 takes the larger expert. Slot capacities
(C0 >= C1) are compiled into the kernel, minimizing padded work while
keeping all 8 cores on identical shapes (SPMD).
"""

import math
from contextlib import ExitStack

import numpy as np

import concourse.bacc as bacc
import concourse.bass as bass
import concourse.mybir as mybir
import concourse.tile as tile
from concourse.bass_utils import run_bass_kernel_spmd

T, D, H, E, TOPK = 4096, 1024, 2048, 16, 2
NOISE_SCALE = 1.0
P = 128
NCORES = 8
EPC = E // NCORES  # experts per core
KD = D // P  # 8  contraction tiles for matmul1
KH = H // P  # 16 contraction tiles for matmul2
DP = D // P  # 8  output-partition blocks for matmul2 (transposed)

# DMA split points (input streaming granularity). The first-processed
# slot streams fine-grained (the PE chases its arrival); later slots are
# never chase-bound and use coarse chunks (fewer transfers = fewer sems).
XT_SPLITS_F = [(0, 2), (2, 4), (4, 6), (6, 8)]  # kd ranges, first slot
W1_SPLITS_F = [(0, 1), (1, 2), (2, 3), (3, 5), (5, 8), (8, 12), (12, 16)]
XT_SPLITS_C = [(0, 8)]
W1_SPLITS_C = [(0, 16)]
W2_SPLITS = [(0, 4), (4, 8)]  # dp ranges
WARMUP_MMS = 8  # N=512 dummy matmuls: pre-warm the PE HAM clock gate

F16 = mybir.dt.float16
F32 = mybir.dt.float32

_CACHE: dict[tuple, bass.Bass] = {}
LAST_RESULTS = None  # BassKernelResults of the most recent run (for profiling)
TRACE = False  # set True (e.g. from test.py) to capture an NTFF trace


def _chunks(C: int) -> list[tuple[int, int]]:
    """Split C columns into near-equal (offset, size) chunks of <=512
    (PSUM bank limit), sizes multiples of 32."""
    out = []
    nch = -(-C // 512)
    c0 = 0
    rem = C
    for i in range(nch):
        n = min(512, rem, -(-(rem // (nch - i)) // 4) * 4)
        out.append((c0, n))
        c0 += n
        rem -= n
    return out


def _build_nc(caps: tuple[int, ...], zero_b1: bool) -> bass.Bass:
    """Bass module for one core: EPC expert FFNs, expert slot e padded to
    caps[e] tokens."""
    CHS = [_chunks(c) for c in caps]

    nc = bacc.Bacc()
    xt_d = [
        nc.declare_dram_parameter(f"xt{e}", [P, KD, caps[e]], F16, isOutput=False)
        for e in range(EPC)
    ]
    w1_d = [
        nc.declare_dram_parameter(f"w1_{e}", [P, KH, KD * P], F16, isOutput=False)
        for e in range(EPC)
    ]
    w2_d = [
        nc.declare_dram_parameter(f"w2_{e}", [P, DP, KH * P], F16, isOutput=False)
        for e in range(EPC)
    ]
    misc_d = (
        None
        if zero_b1
        else nc.declare_dram_parameter("misc", [P, EPC * KH], F32, isOutput=False)
    )
    yt_d = [
        nc.declare_dram_parameter(f"yt{e}", [P, DP, caps[e]], F16, isOutput=True)
        for e in range(EPC)
    ]

    # smaller slot first: shortest critical path to the first matmul, and
    # the larger slot's (smaller-chunked) outputs land on the kernel tail
    ord_slots = sorted(range(EPC), key=lambda s: caps[s])

    with ExitStack() as ctx:
        tc = ctx.enter_context(tile.TileContext(nc))
        in_pool = ctx.enter_context(tc.tile_pool(name="in_pool", bufs=1))
        h_pool = ctx.enter_context(tc.tile_pool(name="h_pool", bufs=1))
        y_pool = ctx.enter_context(tc.tile_pool(name="y_pool", bufs=3))
        ps1_pool = ctx.enter_context(tc.tile_pool(name="ps1_pool", bufs=2, space="PSUM"))
        ps2_pool = ctx.enter_context(tc.tile_pool(name="ps2_pool", bufs=2, space="PSUM"))

        # --- PE pre-warm: dummy matmuls on a zeroed tile flip the HAM
        # clock gate to full rate while the input DMAs stream in; they end
        # before the first real matmul's data lands, so they never block.
        warm = in_pool.tile([P, 512], F16, name="warm", tag="warm")
        nc.vector.memset(warm[:], 0.0)
        wps = ps2_pool.tile([P, 512], F32, name="wps", tag="psy_0")
        for i in range(WARMUP_MMS):
            nc.tensor.matmul(
                wps[:, :], lhsT=warm[:, 0:P], rhs=warm[:, :], start=True, stop=True
            )

        # --- input tiles (kept <=3D: 4D slices degrade PE streaming) ---
        miscs = (
            None
            if zero_b1
            else in_pool.tile([P, EPC * KH], F32, name="miscs", tag="miscs")
        )
        xts = {}
        w1s = {}
        w2s = {}
        hts = {}
        xt_splits = {}
        w1_splits = {}
        for si, s in enumerate(ord_slots):
            C = caps[s]
            xt_splits[s] = XT_SPLITS_F if si == 0 else XT_SPLITS_C
            w1_splits[s] = W1_SPLITS_F if si == 0 else W1_SPLITS_C
            xts[s] = [
                in_pool.tile([P, k1 - k0, C], F16, name=f"xt{s}_{i}", tag=f"xt{s}_{i}")
                for i, (k0, k1) in enumerate(xt_splits[s])
            ]
            w1s[s] = [
                in_pool.tile([P, m1 - m0, KD * P], F16, name=f"w1_{s}_{i}", tag=f"w1_{s}_{i}")
                for i, (m0, m1) in enumerate(w1_splits[s])
            ]
            w2s[s] = [
                in_pool.tile([P, d1 - d0, KH * P], F16, name=f"w2_{s}_{i}", tag=f"w2_{s}_{i}")
                for i, (d0, d1) in enumerate(W2_SPLITS)
            ]
            hts[s] = h_pool.tile([P, KH, C], F16, name=f"hts{s}", tag=f"hts{s}")

        # --- input DMAs on the sync HWDGE ring, in consumption order.
        # FIFO per ring => this order IS the stream priority.
        if not zero_b1:
            nc.sync.dma_start(miscs[:], misc_d[:, :])
        for si, s in enumerate(ord_slots):
            # interleave x and W1 chunks in consumption order so the first
            # matmuls fire early and the mh-chase never stalls long
            xsp, wsp = xt_splits[s], w1_splits[s]
            nxt = min(2, len(xsp))
            for i in range(nxt):
                nc.sync.dma_start(xts[s][i][:], xt_d[s][:, xsp[i][0] : xsp[i][1], :])
                if i < len(wsp):
                    nc.sync.dma_start(w1s[s][i][:], w1_d[s][:, wsp[i][0] : wsp[i][1], :])
            for i in range(nxt, len(xsp)):
                nc.sync.dma_start(xts[s][i][:], xt_d[s][:, xsp[i][0] : xsp[i][1], :])
            for i in range(nxt, len(wsp)):
                nc.sync.dma_start(w1s[s][i][:], w1_d[s][:, wsp[i][0] : wsp[i][1], :])
            for i, (d0, d1) in enumerate(W2_SPLITS):
                nc.sync.dma_start(w2s[s][i][:], w2_d[s][:, d0:d1, :])

        # --- compute ---
        for s in ord_slots:
            C = caps[s]
            chs = CHS[s]

            # matmul1: hT[mh] = relu(sum_kd W1[kd,mh].T @ xT[kd] + b1)
            # chunk loop innermost: consecutive matmuls share lhsT, so the
            # weight load is paid once per (mh, kd)
            for mh in range(KH):
                wsp = w1_splits[s]
                w1i = next(i for i, (m0, m1) in enumerate(wsp) if m0 <= mh < m1)
                w1t = w1s[s][w1i]
                mo = mh - wsp[w1i][0]
                pss = [
                    ps1_pool.tile([P, n], F32, name=f"ps1_{s}_{mh}_{i}", tag=f"ps1_{i}")
                    for i, (_, n) in enumerate(chs)
                ]
                for kd in range(KD):
                    xsp = xt_splits[s]
                    xi = next(i for i, (k0, k1) in enumerate(xsp) if k0 <= kd < k1)
                    ko = kd - xsp[xi][0]
                    for i, (c0, n) in enumerate(chs):
                        nc.tensor.matmul(
                            pss[i][:, :],
                            lhsT=w1t[:, mo, kd * P : (kd + 1) * P],
                            rhs=xts[s][xi][:, ko, c0 : c0 + n],
                            start=(kd == 0),
                            stop=(kd == KD - 1),
                        )
                for i, (c0, n) in enumerate(chs):
                    nc.scalar.activation(
                        hts[s][:, mh, c0 : c0 + n],
                        pss[i][:, :],
                        mybir.ActivationFunctionType.Relu,
                        bias=(
                            0.0
                            if zero_b1
                            else miscs[:, s * KH + mh : s * KH + mh + 1]
                        ),
                    )

            # matmul2 (transposed): yT[dp] = sum_kh W2[kh,dp].T @ hT[kh]
            for dp in range(DP):
                w2i = next(i for i, (d0, d1) in enumerate(W2_SPLITS) if d0 <= dp < d1)
                w2t = w2s[s][w2i]
                do = dp - W2_SPLITS[w2i][0]
                psys = [
                    ps2_pool.tile([P, n], F32, name=f"psy_{s}_{dp}_{i}", tag=f"psy_{i}")
                    for i, (_, n) in enumerate(chs)
                ]
                for kh in range(KH):
                    for i, (c0, n) in enumerate(chs):
                        nc.tensor.matmul(
                            psys[i][:, :],
                            lhsT=w2t[:, do, kh * P : (kh + 1) * P],
                            rhs=hts[s][:, kh, c0 : c0 + n],
                            start=(kh == 0),
                            stop=(kh == KH - 1),
                        )
                last_group = s == ord_slots[-1] and dp == DP - 1
                ys = y_pool.tile([P, C], F16, name=f"ys_{s}_{dp}", tag="ys")
                for i, (c0, n) in enumerate(chs):
                    if last_group and i == len(chs) - 1 and len(chs) > 1:
                        # kernel tail: cast the final chunk on ScalarE, in
                        # parallel with VectorE's cast of the previous one
                        nc.scalar.activation(
                            ys[:, c0 : c0 + n],
                            psys[i][:, :],
                            mybir.ActivationFunctionType.Copy,
                        )
                    else:
                        nc.vector.tensor_copy(ys[:, c0 : c0 + n], psys[i][:, :])
                # one store per dp group; outputs ride the scalar HWDGE
                # ring, never queued behind the input stream on sync
                eng = nc.sync if last_group else nc.scalar
                eng.dma_start(yt_d[s][:, dp, :], ys[:, :])

    nc.compile()
    return nc


def _route(x, noise_eps, Wg, Wn):
    """Replicate the reference noisy top-2 gating on host (fp64)."""
    xl = x.astype(np.float64)
    logits = xl @ Wg.astype(np.float64).T + NOISE_SCALE * noise_eps.astype(
        np.float64
    ) * np.logaddexp(0.0, xl @ Wn.astype(np.float64).T)
    # jax.lax.top_k: values sorted descending, ties broken by lower index
    top_idx = np.argsort(-logits, axis=1, kind="stable")[:, :TOPK]
    tv = np.take_along_axis(logits, top_idx, axis=1)
    ex = np.exp(tv - tv.max(axis=1, keepdims=True))
    gates = ex / ex.sum(axis=1, keepdims=True)
    return top_idx, gates.astype(np.float32)


def kernel(x, noise_eps, Wg, Wn, W1, b1, W2, b2):
    global LAST_RESULTS
    # inputs may arrive as jax arrays; force plain numpy so all host math
    # (routing, gather/scatter) stays off-device
    x = np.ascontiguousarray(np.asarray(x), np.float32)
    noise_eps = np.asarray(noise_eps, np.float32)
    Wg = np.asarray(Wg, np.float32)
    Wn = np.asarray(Wn, np.float32)
    W1 = np.asarray(W1, np.float32)
    b1 = np.asarray(b1, np.float32)
    W2 = np.asarray(W2, np.float32)
    b2 = np.asarray(b2, np.float32)

    top_idx, gates = _route(x, noise_eps, Wg, Wn)

    # token lists per expert
    tok_lists = [np.nonzero((top_idx == e).any(axis=1))[0] for e in range(E)]
    counts = np.array([len(t) for t in tok_lists])

    # pair largest with smallest; slot 0 = larger expert of each pair
    order = np.argsort(-counts, kind="stable")
    slot_expert = np.zeros((NCORES, EPC), np.int64)  # (core, slot) -> expert
    for c in range(NCORES):
        slot_expert[c, 0] = order[c]
        slot_expert[c, 1] = order[E - 1 - c]
    cap = lambda n: max(64, int(math.ceil(n / 4) * 4))
    caps = tuple(
        cap(int(counts[slot_expert[:, s]].max())) for s in range(EPC)
    )  # per-slot capacity, uniform across cores

    zero_b1 = not np.any(b1)
    key = (caps, zero_b1)
    nc = _CACHE.get(key)
    if nc is None:
        nc = _CACHE[key] = _build_nc(caps, zero_b1)

    x16 = x.astype(np.float16)
    W1_16 = np.asarray(W1, np.float16)
    W2_16 = np.asarray(W2, np.float16)
    b1f = np.asarray(b1, np.float32)

    # position of (t, k) within its expert's batch
    pos_of = np.zeros((T, TOPK), np.int64)

    in_maps = []
    for c in range(NCORES):
        m = {}
        misc_np = None if zero_b1 else np.zeros((P, EPC * KH), np.float32)
        for s in range(EPC):
            e = int(slot_expert[c, s])
            C = caps[s]
            toks = tok_lists[e]
            xt_np = np.zeros((P, KD, C), np.float16)
            xt_np[:, :, : len(toks)] = x16[toks].T.reshape(KD, P, -1).transpose(1, 0, 2)
            m[f"xt{s}"] = xt_np
            m[f"w1_{s}"] = np.ascontiguousarray(
                W1_16[e].reshape(KD, P, KH, P).transpose(1, 2, 0, 3)
            ).reshape(P, KH, KD * P)
            m[f"w2_{s}"] = np.ascontiguousarray(
                W2_16[e].reshape(KH, P, DP, P).transpose(1, 2, 0, 3)
            ).reshape(P, DP, KH * P)
            k_sel = (top_idx[toks] == e).argmax(axis=1)
            pos_of[toks, k_sel] = np.arange(len(toks))
            if not zero_b1:
                misc_np[:, s * KH : (s + 1) * KH] = b1f[e].reshape(KH, P).T
        if not zero_b1:
            m["misc"] = misc_np
        in_maps.append(m)

    res = run_bass_kernel_spmd(nc, in_maps, core_ids=list(range(NCORES)), trace=TRACE)
    LAST_RESULTS = res

    # Y[e] = UNSCALED outputs of expert e as [C, D] (from transposed [P,DP,C])
    Cmax = max(caps)
    Yall = np.zeros((E, Cmax, D), np.float16)
    for c in range(NCORES):
        for s in range(EPC):
            e = int(slot_expert[c, s])
            yt = res.results[c][f"yt{s}"]  # [P, DP, C] f16
            Yall[e, : caps[s]] = yt.transpose(2, 1, 0).reshape(caps[s], D)

    # weighted combine (gates + b2 applied on host)
    y0 = Yall[top_idx[:, 0], pos_of[:, 0]].astype(np.float32)
    y1 = Yall[top_idx[:, 1], pos_of[:, 1]].astype(np.float32)
    out = gates[:, 0:1] * y0 + gates[:, 1:2] * y1
    b2f = np.asarray(b2, np.float32)
    out += gates[:, 0:1] * b2f[top_idx[:, 0]] + gates[:, 1:2] * b2f[top_idx[:, 1]]
    return out.astype(np.float32)
